# revision 4
# baseline (speedup 1.0000x reference)
"""Haar DWT edge-magnitude kernel for TRN2 (8 NeuronCores, SPMD) — v3.

Math (see kernel.py): with R = x[h]-x[h+1], P = x[h]+x[h+1] (vertical),
q = P[w]-P[w+1] (horizontal),
    out^2 = 0.5*R[w]^2 + 0.5*R[w+1]^2 + 0.25*q^2
and out(w=W-1) == out(w=W-2) exactly (reflect pad).

Engine split per chunk (all bf16 except PSUM/output):
- Pool:  SWDGE cast-DMA load fp32->bf16; last-column fixup copy.
- DVE:   r = t - t+W, p = t + t+W, q = p - p+1, rr = r*r   (2x 16-bit mode)
- Act:   qq = Square(0.5*q) = 0.25 q^2 ; final out = Sqrt(PSUM)
- PE:    u(PSUM) = mm(0.5I, rr[w]) + mm(0.5I, rr[w+1]) + mm(I, qq)
- SP:    HWDGE store out (fp32)
"""

import numpy as np

import concourse.bass as bass
from concourse import bacc, mybir, tile
from concourse.bass_utils import run_bass_kernel_spmd

AF = mybir.ActivationFunctionType
OP = mybir.AluOpType
FP32 = mybir.dt.float32
BF16 = mybir.dt.bfloat16

B, C, H, W = 8, 64, 256, 256
NCORES = 8
P = 128                   # SBUF partitions: 64 images x 2 halves
RH = H // 2               # rows per half
FREE = RH * W             # 32768 output elements per partition
ROW_PLAN = [2, 2, 4] + [8] * 14 + [4, 2, 2]   # sums to RH=128
assert sum(ROW_PLAN) == RH
FMAXC = 8 * W
PSW = 512                 # PSUM bank width in fp32


def _make_scaled_identity(nc, ap, fill):
    nc.gpsimd.memset(ap, 0.0)
    nc.gpsimd.affine_select(
        out=ap,
        in_=ap,
        compare_op=OP.not_equal,
        fill=fill,
        base=0,
        pattern=[[-1, 128]],
        channel_multiplier=1,
    )


def build_nc(reps: int = 1):
    nc = bacc.Bacc("TRN2", target_bir_lowering=False)
    xd = nc.dram_tensor("x", [P, FREE + W], FP32, kind="ExternalInput")
    od = nc.dram_tensor("out", [P, FREE], FP32, kind="ExternalOutput")

    with tile.TileContext(nc) as tc:
        with (
            tc.tile_pool(name="const", bufs=1) as const_pool,
            tc.tile_pool(name="io", bufs=3) as io_pool,
            tc.tile_pool(name="tmp", bufs=2) as tmp_pool,
            tc.tile_pool(name="ps", bufs=2, space="PSUM") as ps_pool,
        ):
            ident_h = const_pool.tile([128, 128], BF16, tag="ih")   # 0.5*I
            ident_1 = const_pool.tile([128, 128], BF16, tag="i1")   # I
            _make_scaled_identity(nc, ident_h[:], 0.5)
            _make_scaled_identity(nc, ident_1[:], 1.0)

            for _rep in range(reps):
                base = 0
                for k, rows in enumerate(ROW_PLAN):
                    F = rows * W
                    # Cast-load F + W elements as bf16; zero the one extra
                    # column so stale SBUF can't inject NaN (its products only
                    # feed outputs the w=W-1 fixup overwrites anyway).
                    t = io_pool.tile([P, FMAXC + W + 1], BF16, tag="in")
                    nc.vector.memset(t[:, F + W : F + W + 1], 0.0)
                    nc.gpsimd.dma_start(t[:, 0 : F + W], xd[:, base : base + F + W])

                    r_t = tmp_pool.tile([P, FMAXC + 1], BF16, tag="r")
                    p_t = tmp_pool.tile([P, FMAXC + 1], BF16, tag="p")
                    nc.vector.tensor_tensor(
                        r_t[:, 0 : F + 1], t[:, 0 : F + 1], t[:, W : F + W + 1], OP.subtract
                    )
                    nc.vector.tensor_tensor(
                        p_t[:, 0 : F + 1], t[:, 0 : F + 1], t[:, W : F + W + 1], OP.add
                    )
                    q = tmp_pool.tile([P, FMAXC], BF16, tag="q")
                    nc.vector.tensor_tensor(
                        q[:, 0:F], p_t[:, 0:F], p_t[:, 1 : F + 1], OP.subtract
                    )
                    # rr: split columns between DVE (3/4) and Act Square (1/4)
                    rr = tmp_pool.tile([P, FMAXC + 1], BF16, tag="rr")
                    rs = (F * 3 // 4) & ~63
                    nc.vector.tensor_tensor(
                        rr[:, 0:rs], r_t[:, 0:rs], r_t[:, 0:rs], OP.mult
                    )
                    nc.scalar.activation(
                        rr[:, rs : F + 1], r_t[:, rs : F + 1], AF.Square
                    )
                    qq = tmp_pool.tile([P, FMAXC], BF16, tag="qq")
                    nc.scalar.activation(qq[:, 0:F], q[:, 0:F], AF.Square, scale=0.5)

                    o = io_pool.tile([P, FMAXC], FP32, tag="out", bufs=4)
                    for j in range(0, F, 2 * PSW):
                        wdt = min(2 * PSW, F - j)
                        u = ps_pool.tile([P, 2 * PSW], FP32, tag="u")
                        for jj in range(0, wdt, PSW):
                            nc.tensor.matmul(
                                u[:, jj : jj + PSW], ident_h[:],
                                rr[:, j + jj : j + jj + PSW],
                                start=True, stop=False,
                            )
                            nc.tensor.matmul(
                                u[:, jj : jj + PSW], ident_h[:],
                                rr[:, j + jj + 1 : j + jj + PSW + 1],
                                start=False, stop=False,
                            )
                            nc.tensor.matmul(
                                u[:, jj : jj + PSW], ident_1[:],
                                qq[:, j + jj : j + jj + PSW],
                                start=False, stop=True,
                            )
                        nc.scalar.activation(o[:, j : j + wdt], u[:, 0:wdt], AF.Sqrt)

                    # out(w=W-1) == out(w=W-2): post-sqrt fixup
                    nc.vector.tensor_copy(
                        o[:, W - 1 : F : W], o[:, W - 2 : F : W]
                    )
                    nc.sync.dma_start(od[:, base : base + F], o[:, 0:F])
                    base += F
    nc.compile()
    return nc


def shard_input(x: np.ndarray) -> list[np.ndarray]:
    """(B,C,H,W) f32 -> per-core [P, FREE+W] arrays with halo appended."""
    xr = np.ascontiguousarray(x, dtype=np.float32).reshape(B * C, 2, RH, W)
    shards = []
    per = (B * C) // NCORES
    for i in range(NCORES):
        xc = xr[i * per : (i + 1) * per]          # (64, 2, RH, W)
        main = xc.reshape(P, FREE)
        halo = np.stack([xc[:, 1, 0, :], xc[:, 1, RH - 2, :]], axis=1)
        arr = np.concatenate([main, halo.reshape(P, W)], axis=1)
        shards.append(np.ascontiguousarray(arr))
    return shards


def unshard_output(outs: list[np.ndarray]) -> np.ndarray:
    per = (B * C) // NCORES
    full = np.empty((B * C, H, W), dtype=np.float32)
    for i, o in enumerate(outs):
        full[i * per : (i + 1) * per] = o.reshape(per, H, W)
    return full.reshape(B, C, H, W)


def kernel(x: np.ndarray) -> np.ndarray:
    nc = build_nc()
    in_maps = [{"x": s} for s in shard_input(x)]
    res = run_bass_kernel_spmd(nc, in_maps, core_ids=list(range(NCORES)))
    return unshard_output([r["out"] for r in res.results])


# revision 6
# speedup vs baseline: 1.3043x; 1.3043x over previous
"""Haar DWT edge-magnitude kernel for TRN2 (8 NeuronCores, SPMD) — v3.

Math (see kernel.py): with R = x[h]-x[h+1], P = x[h]+x[h+1] (vertical),
q = P[w]-P[w+1] (horizontal),
    out^2 = 0.5*R[w]^2 + 0.5*R[w+1]^2 + 0.25*q^2
and out(w=W-1) == out(w=W-2) exactly (reflect pad).

Engine split per chunk (all bf16 except PSUM/output):
- Pool:  SWDGE cast-DMA load fp32->bf16; last-column fixup copy.
- DVE:   r = t - t+W, p = t + t+W, q = p - p+1, rr = r*r   (2x 16-bit mode)
- Act:   qq = Square(0.5*q) = 0.25 q^2 ; final out = Sqrt(PSUM)
- PE:    u(PSUM) = mm(0.5I, rr[w]) + mm(0.5I, rr[w+1]) + mm(I, qq)
- SP:    HWDGE store out (fp32)
"""

import numpy as np

import concourse.bass as bass
from concourse import bacc, mybir, tile
from concourse.bass_utils import run_bass_kernel_spmd

AF = mybir.ActivationFunctionType
OP = mybir.AluOpType
FP32 = mybir.dt.float32
BF16 = mybir.dt.bfloat16

B, C, H, W = 8, 64, 256, 256
NCORES = 8
P = 128                   # SBUF partitions: 64 images x 2 halves
RH = H // 2               # rows per half
FREE = RH * W             # 32768 output elements per partition
ROW_PLAN = [16] * 8   # sums to RH=128
assert sum(ROW_PLAN) == RH
FMAXC = 16 * W
PSW = 512                 # PSUM bank width in fp32


def _make_scaled_identity(nc, ap, fill):
    nc.gpsimd.memset(ap, 0.0)
    nc.gpsimd.affine_select(
        out=ap,
        in_=ap,
        compare_op=OP.not_equal,
        fill=fill,
        base=0,
        pattern=[[-1, 128]],
        channel_multiplier=1,
    )


def build_nc(reps: int = 1):
    nc = bacc.Bacc("TRN2", target_bir_lowering=False)
    xd = nc.dram_tensor("x", [P, FREE + W], FP32, kind="ExternalInput")
    od = nc.dram_tensor("out", [P, FREE], FP32, kind="ExternalOutput")

    with tile.TileContext(nc) as tc:
        with (
            tc.tile_pool(name="const", bufs=1) as const_pool,
            tc.tile_pool(name="io", bufs=3) as io_pool,
            tc.tile_pool(name="tmp", bufs=2) as tmp_pool,
            tc.tile_pool(name="ps", bufs=2, space="PSUM") as ps_pool,
        ):
            ident_h = const_pool.tile([128, 128], BF16, tag="ih")   # 0.5*I
            ident_1 = const_pool.tile([128, 128], BF16, tag="i1")   # I
            _make_scaled_identity(nc, ident_h[:], 0.5)
            _make_scaled_identity(nc, ident_1[:], 1.0)

            for _rep in range(reps):
                base = 0
                for k, rows in enumerate(ROW_PLAN):
                    F = rows * W
                    # Cast-load F + W elements as bf16; zero the one extra
                    # column so stale SBUF can't inject NaN (its products only
                    # feed outputs the w=W-1 fixup overwrites anyway).
                    t = io_pool.tile([P, FMAXC + W + 1], BF16, tag="in")
                    nc.vector.memset(t[:, F + W : F + W + 1], 0.0)
                    nc.gpsimd.dma_start(t[:, 0 : F + W], xd[:, base : base + F + W])

                    r_t = tmp_pool.tile([P, FMAXC + 1], BF16, tag="r")
                    p_t = tmp_pool.tile([P, FMAXC + 1], BF16, tag="p")
                    nc.vector.tensor_tensor(
                        r_t[:, 0 : F + 1], t[:, 0 : F + 1], t[:, W : F + W + 1], OP.subtract
                    )
                    nc.vector.tensor_tensor(
                        p_t[:, 0 : F + 1], t[:, 0 : F + 1], t[:, W : F + W + 1], OP.add
                    )
                    q = tmp_pool.tile([P, FMAXC], BF16, tag="q")
                    nc.vector.tensor_tensor(
                        q[:, 0:F], p_t[:, 0:F], p_t[:, 1 : F + 1], OP.subtract
                    )
                    # rr: split columns between DVE (3/4) and Act Square (1/4)
                    rr = tmp_pool.tile([P, FMAXC + 1], BF16, tag="rr")
                    rs = (F * 3 // 4) & ~63
                    nc.vector.tensor_tensor(
                        rr[:, 0:rs], r_t[:, 0:rs], r_t[:, 0:rs], OP.mult
                    )
                    nc.scalar.activation(
                        rr[:, rs : F + 1], r_t[:, rs : F + 1], AF.Square
                    )
                    qq = tmp_pool.tile([P, FMAXC], BF16, tag="qq")
                    nc.scalar.activation(qq[:, 0:F], q[:, 0:F], AF.Square, scale=0.5)

                    o = io_pool.tile([P, FMAXC], FP32, tag="out", bufs=4)
                    for j in range(0, F, 2 * PSW):
                        wdt = min(2 * PSW, F - j)
                        u = ps_pool.tile([P, 2 * PSW], FP32, tag="u")
                        for jj in range(0, wdt, PSW):
                            nc.tensor.matmul(
                                u[:, jj : jj + PSW], ident_h[:],
                                rr[:, j + jj : j + jj + PSW],
                                start=True, stop=False,
                            )
                            nc.tensor.matmul(
                                u[:, jj : jj + PSW], ident_h[:],
                                rr[:, j + jj + 1 : j + jj + PSW + 1],
                                start=False, stop=False,
                            )
                            nc.tensor.matmul(
                                u[:, jj : jj + PSW], ident_1[:],
                                qq[:, j + jj : j + jj + PSW],
                                start=False, stop=True,
                            )
                        nc.scalar.activation(o[:, j : j + wdt], u[:, 0:wdt], AF.Sqrt)

                    # out(w=W-1) == out(w=W-2): post-sqrt fixup
                    nc.vector.tensor_copy(
                        o[:, W - 1 : F : W], o[:, W - 2 : F : W]
                    )
                    nc.sync.dma_start(od[:, base : base + F], o[:, 0:F])
                    base += F
    nc.compile()
    return nc


def shard_input(x: np.ndarray) -> list[np.ndarray]:
    """(B,C,H,W) f32 -> per-core [P, FREE+W] arrays with halo appended."""
    xr = np.ascontiguousarray(x, dtype=np.float32).reshape(B * C, 2, RH, W)
    shards = []
    per = (B * C) // NCORES
    for i in range(NCORES):
        xc = xr[i * per : (i + 1) * per]          # (64, 2, RH, W)
        main = xc.reshape(P, FREE)
        halo = np.stack([xc[:, 1, 0, :], xc[:, 1, RH - 2, :]], axis=1)
        arr = np.concatenate([main, halo.reshape(P, W)], axis=1)
        shards.append(np.ascontiguousarray(arr))
    return shards


def unshard_output(outs: list[np.ndarray]) -> np.ndarray:
    per = (B * C) // NCORES
    full = np.empty((B * C, H, W), dtype=np.float32)
    for i, o in enumerate(outs):
        full[i * per : (i + 1) * per] = o.reshape(per, H, W)
    return full.reshape(B, C, H, W)


def kernel(x: np.ndarray) -> np.ndarray:
    nc = build_nc()
    in_maps = [{"x": s} for s in shard_input(x)]
    res = run_bass_kernel_spmd(nc, in_maps, core_ids=list(range(NCORES)))
    return unshard_output([r["out"] for r in res.results])


# revision 7
# speedup vs baseline: 1.4187x; 1.0877x over previous
"""Haar DWT edge-magnitude kernel for TRN2 (8 NeuronCores, SPMD) — v3.

Math (see kernel.py): with R = x[h]-x[h+1], P = x[h]+x[h+1] (vertical),
q = P[w]-P[w+1] (horizontal),
    out^2 = 0.5*R[w]^2 + 0.5*R[w+1]^2 + 0.25*q^2
and out(w=W-1) == out(w=W-2) exactly (reflect pad).

Engine split per chunk (all bf16 except PSUM/output):
- Pool:  SWDGE cast-DMA load fp32->bf16; last-column fixup copy.
- DVE:   r = t - t+W, p = t + t+W, q = p - p+1, rr = r*r   (2x 16-bit mode)
- Act:   qq = Square(0.5*q) = 0.25 q^2 ; final out = Sqrt(PSUM)
- PE:    u(PSUM) = mm(0.5I, rr[w]) + mm(0.5I, rr[w+1]) + mm(I, qq)
- SP:    HWDGE store out (fp32)
"""

import numpy as np

import concourse.bass as bass
from concourse import bacc, mybir, tile
from concourse.bass_utils import run_bass_kernel_spmd

AF = mybir.ActivationFunctionType
OP = mybir.AluOpType
FP32 = mybir.dt.float32
BF16 = mybir.dt.bfloat16

B, C, H, W = 8, 64, 256, 256
NCORES = 8
P = 128                   # SBUF partitions: 64 images x 2 halves
RH = H // 2               # rows per half
FREE = RH * W             # 32768 output elements per partition
ROW_PLAN = [24] * 5 + [8]   # sums to RH=128
assert sum(ROW_PLAN) == RH
FMAXC = 24 * W
PSW = 512                 # PSUM bank width in fp32


def _make_scaled_identity(nc, ap, fill):
    nc.gpsimd.memset(ap, 0.0)
    nc.gpsimd.affine_select(
        out=ap,
        in_=ap,
        compare_op=OP.not_equal,
        fill=fill,
        base=0,
        pattern=[[-1, 128]],
        channel_multiplier=1,
    )


def build_nc(reps: int = 1):
    nc = bacc.Bacc("TRN2", target_bir_lowering=False)
    xd = nc.dram_tensor("x", [P, FREE + W], FP32, kind="ExternalInput")
    od = nc.dram_tensor("out", [P, FREE], BF16, kind="ExternalOutput")

    with tile.TileContext(nc) as tc:
        with (
            tc.tile_pool(name="const", bufs=1) as const_pool,
            tc.tile_pool(name="io", bufs=3) as io_pool,
            tc.tile_pool(name="tmp", bufs=2) as tmp_pool,
            tc.tile_pool(name="ps", bufs=2, space="PSUM") as ps_pool,
        ):
            ident_h = const_pool.tile([128, 128], BF16, tag="ih")   # 0.5*I
            ident_1 = const_pool.tile([128, 128], BF16, tag="i1")   # I
            _make_scaled_identity(nc, ident_h[:], 0.5)
            _make_scaled_identity(nc, ident_1[:], 1.0)

            for _rep in range(reps):
                base = 0
                for k, rows in enumerate(ROW_PLAN):
                    F = rows * W
                    # Cast-load F + W elements as bf16; zero the one extra
                    # column so stale SBUF can't inject NaN (its products only
                    # feed outputs the w=W-1 fixup overwrites anyway).
                    t = io_pool.tile([P, FMAXC + W + 1], BF16, tag="in")
                    nc.vector.memset(t[:, F + W : F + W + 1], 0.0)
                    nc.gpsimd.dma_start(t[:, 0 : F + W], xd[:, base : base + F + W])

                    r_t = tmp_pool.tile([P, FMAXC + 1], BF16, tag="r")
                    p_t = tmp_pool.tile([P, FMAXC + 1], BF16, tag="p")
                    nc.vector.tensor_tensor(
                        r_t[:, 0 : F + 1], t[:, 0 : F + 1], t[:, W : F + W + 1], OP.subtract
                    )
                    nc.vector.tensor_tensor(
                        p_t[:, 0 : F + 1], t[:, 0 : F + 1], t[:, W : F + W + 1], OP.add
                    )
                    q = tmp_pool.tile([P, FMAXC], BF16, tag="q")
                    nc.vector.tensor_tensor(
                        q[:, 0:F], p_t[:, 0:F], p_t[:, 1 : F + 1], OP.subtract
                    )
                    # rr: split columns between DVE (3/4) and Act Square (1/4)
                    rr = tmp_pool.tile([P, FMAXC + 1], BF16, tag="rr")
                    rs = (F * 3 // 4) & ~63
                    nc.vector.tensor_tensor(
                        rr[:, 0:rs], r_t[:, 0:rs], r_t[:, 0:rs], OP.mult
                    )
                    nc.scalar.activation(
                        rr[:, rs : F + 1], r_t[:, rs : F + 1], AF.Square
                    )
                    qq = tmp_pool.tile([P, FMAXC], BF16, tag="qq")
                    nc.scalar.activation(qq[:, 0:F], q[:, 0:F], AF.Square, scale=0.5)

                    o = io_pool.tile([P, FMAXC], BF16, tag="out", bufs=3)
                    for j in range(0, F, 2 * PSW):
                        wdt = min(2 * PSW, F - j)
                        u = ps_pool.tile([P, 2 * PSW], FP32, tag="u")
                        for jj in range(0, wdt, PSW):
                            nc.tensor.matmul(
                                u[:, jj : jj + PSW], ident_h[:],
                                rr[:, j + jj : j + jj + PSW],
                                start=True, stop=False,
                            )
                            nc.tensor.matmul(
                                u[:, jj : jj + PSW], ident_h[:],
                                rr[:, j + jj + 1 : j + jj + PSW + 1],
                                start=False, stop=False,
                            )
                            nc.tensor.matmul(
                                u[:, jj : jj + PSW], ident_1[:],
                                qq[:, j + jj : j + jj + PSW],
                                start=False, stop=True,
                            )
                        nc.scalar.activation(o[:, j : j + wdt], u[:, 0:wdt], AF.Sqrt)

                    # out(w=W-1) == out(w=W-2): post-sqrt fixup
                    nc.vector.tensor_copy(
                        o[:, W - 1 : F : W], o[:, W - 2 : F : W]
                    )
                    nc.sync.dma_start(od[:, base : base + F], o[:, 0:F])
                    base += F
    nc.compile()
    return nc


def shard_input(x: np.ndarray) -> list[np.ndarray]:
    """(B,C,H,W) f32 -> per-core [P, FREE+W] arrays with halo appended."""
    xr = np.ascontiguousarray(x, dtype=np.float32).reshape(B * C, 2, RH, W)
    shards = []
    per = (B * C) // NCORES
    for i in range(NCORES):
        xc = xr[i * per : (i + 1) * per]          # (64, 2, RH, W)
        main = xc.reshape(P, FREE)
        halo = np.stack([xc[:, 1, 0, :], xc[:, 1, RH - 2, :]], axis=1)
        arr = np.concatenate([main, halo.reshape(P, W)], axis=1)
        shards.append(np.ascontiguousarray(arr))
    return shards


def unshard_output(outs: list[np.ndarray]) -> np.ndarray:
    per = (B * C) // NCORES
    full = np.empty((B * C, H, W), dtype=np.float32)
    for i, o in enumerate(outs):
        full[i * per : (i + 1) * per] = o.astype(np.float32).reshape(per, H, W)
    return full.reshape(B, C, H, W)


def kernel(x: np.ndarray) -> np.ndarray:
    nc = build_nc()
    in_maps = [{"x": s} for s in shard_input(x)]
    res = run_bass_kernel_spmd(nc, in_maps, core_ids=list(range(NCORES)))
    return unshard_output([r["out"] for r in res.results])
